# revision 31
# baseline (speedup 1.0000x reference)
import numpy as np

import concourse.bass as bass
import concourse.mybir as mybir
import concourse.tile as tile
from concourse.bass_utils import run_bass_kernel_spmd
from concourse.vector_clock import ScopedClock

_MAX_WAITS = 1


def _split_drain_and_barrier(self, tick_clock, wait_clock):
    import bass_rust

    drain_inst = self.nc.sync.drain()
    wait_clock.add_sem_waits(
        drain_inst.ins, ScopedClock({None: tick_clock.global_clock})
    )
    si = drain_inst.ins.sync_info
    waits = list(si.on_wait)
    if len(waits) > _MAX_WAITS:
        si.on_wait = waits[:_MAX_WAITS]
        drain_inst.ins.sync_info = si
        for k in range(_MAX_WAITS, len(waits), _MAX_WAITS):
            extra = self.nc.sync.drain()
            esi = extra.ins.sync_info
            if esi is None:
                esi = bass_rust.SyncInfo(
                    on_wait=waits[k : k + _MAX_WAITS], on_update=[]
                )
            else:
                esi.on_wait = waits[k : k + _MAX_WAITS]
            extra.ins.sync_info = esi

    self.nc.all_engine_barrier()
    assert self.sems is not None
    popped = self.nc._tile_sem_poison_stack.pop()
    assert popped is self._sem_poison
    self.nc.clear_and_free_semaphores(list(self.sems.allocated().values()))
    self.nc.all_engine_barrier()


tile.TileContext._drain_and_barrier = _split_drain_and_barrier

_orig_lower_ordered = tile.TileContext._lower_ordered_insts


def _split_waits_lower(self, ordered):
    import bass_rust

    for bb, insts in ordered.items():
        new = []
        for inst in insts:
            si = getattr(inst, "sync_info", None)
            waits = list(si.on_wait) if si is not None else []
            if len(waits) > _MAX_WAITS:
                eng = inst.engine
                for w in waits[:-_MAX_WAITS]:
                    carrier = self.nc.engines[eng].drain(fusable=False).ins
                    csi = carrier.sync_info
                    if csi is None:
                        csi = bass_rust.SyncInfo(on_wait=[w], on_update=[])
                    else:
                        csi.on_wait = [w]
                    carrier.sync_info = csi
                    new.append(carrier)
                si.on_wait = waits[-_MAX_WAITS:]
                inst.sync_info = si
            new.append(inst)
        insts[:] = new
    return _orig_lower_ordered(self, ordered)


tile.TileContext._lower_ordered_insts = _split_waits_lower

F32 = mybir.dt.float32
I16 = mybir.dt.int16
BF16 = mybir.dt.bfloat16

B = 8
HH = 32
N = 4096
C = 64
NROWS = 32768
NB = 8
MC = 32

SCH_A = 184.6650
SCH_B = 16248.58

DVE_EXP_EVERY = 3
GPSIMD_RESID = True


def build_kernel(dve_exp_every=DVE_EXP_EVERY, gpsimd_resid=GPSIMD_RESID,
                 phases=99):
    nc = bass.Bass()

    x = nc.declare_dram_parameter("x", [NROWS, C], F32, isOutput=False)
    wf = nc.declare_dram_parameter("wf", [65, 8], BF16, isOutput=False)
    wg = nc.declare_dram_parameter("wg", [65, 8], BF16, isOutput=False)
    wh = nc.declare_dram_parameter("wh", [65, 33], BF16, isOutput=False)
    wv = nc.declare_dram_parameter("wv", [33, C], BF16, isOutput=False)
    ident = nc.declare_dram_parameter("ident", [128, 128], F32, isOutput=False)
    out = nc.declare_dram_parameter("out", [NROWS, C], F32, isOutput=True)

    x_sub = x.rearrange(
        "(h2 hb w2 wb d2 db) c -> hb wb db h2 w2 d2 c",
        h2=16, hb=2, w2=16, wb=2, d2=16, db=2,
    )[0, 0, 0]

    def xs_chunk(mc):
        return x_sub[mc >> 1, 8 * (mc & 1) : 8 * (mc & 1) + 8]

    with tile.TileContext(nc) as tc:
        with (
            tc.tile_pool(name="const", bufs=1) as const_pool,
            tc.tile_pool(name="persist", bufs=1) as persist,
            tc.tile_pool(name="stage", bufs=6) as stage,
            tc.tile_pool(name="et", bufs=6) as et_pool,
            tc.tile_pool(name="xres", bufs=4) as xres_pool,
            tc.tile_pool(name="ores", bufs=2) as ores_pool,
            tc.tile_pool(name="p2", bufs=3, space=bass.MemorySpace.PSUM) as p2,
            tc.tile_pool(name="p1", bufs=2, space=bass.MemorySpace.PSUM) as p1,
        ):
            id_sb = const_pool.tile([128, 128], F32)
            nc.sync.dma_start(id_sb[:], ident[:])
            wf_sb = const_pool.tile([65, 8], BF16)
            nc.sync.dma_start(wf_sb[:], wf[:])
            wg_sb = const_pool.tile([65, 8], BF16)
            nc.sync.dma_start(wg_sb[:], wg[:])
            wh_sb = const_pool.tile([65, 33], BF16)
            nc.sync.dma_start(wh_sb[:], wh[:])
            wv_sb = const_pool.tile([33, C], BF16)
            nc.sync.dma_start(wv_sb[:], wv[:])
            ones_sb = const_pool.tile([1, 1], F32)
            nc.vector.memset(ones_sb[:], 1.0)

            xsT = persist.tile([65, N], BF16)
            nc.vector.memset(xsT[64:65, :], 1.0)
            for mc in range(MC):
                st = stage.tile([128, C], F32, tag="xs_stage")
                nc.sync.dma_start(st[:], xs_chunk(mc))
                tp = p1.tile([64, 128], F32, tag="oT")
                nc.tensor.matmul(tp[:], st[:], id_sb[:], start=True, stop=True)
                nc.scalar.copy(xsT[0:64, mc * 128 : (mc + 1) * 128], tp[:])

            if phases < 2:
                return nc

            fT = persist.tile([8, N], BF16)
            gT = persist.tile([8, N], BF16)
            for nb in range(NB):
                pf = p1.tile([8, 512], F32, tag="oT")
                nc.tensor.matmul(
                    pf[:], wf_sb[:], xsT[:, nb * 512 : (nb + 1) * 512],
                    start=True, stop=True,
                )
                nc.vector.tensor_copy(fT[:, nb * 512 : (nb + 1) * 512], pf[:])
                pg = p1.tile([8, 512], F32, tag="oT")
                nc.tensor.matmul(
                    pg[:], wg_sb[:], xsT[:, nb * 512 : (nb + 1) * 512],
                    start=True, stop=True,
                )
                nc.vector.tensor_copy(gT[:, nb * 512 : (nb + 1) * 512], pg[:])
            h_aug = persist.tile([128, MC * 33], BF16)
            for mc in range(MC):
                ph = p1.tile([128, 33], F32, tag="oT")
                nc.tensor.matmul(
                    ph[:], xsT[:, mc * 128 : (mc + 1) * 128], wh_sb[:],
                    start=True, stop=True,
                )
                nc.scalar.copy(h_aug[:, mc * 33 : (mc + 1) * 33], ph[:])

            if phases < 3:
                return nc

            vT = persist.tile([C, N], F32)
            inv_nat = persist.tile([128, MC], F32)
            v_nat = persist.tile([128, MC * C], F32)
            vscratch = nc.dram_tensor("vscratch", [N, C], F32)
            vsc_w = vscratch.rearrange("(vc p) c -> p vc c", p=128)

            for nb in range(NB):
                oT = p1.tile([33, 512], F32, tag="oT")
                for mc2 in range(MC // 2):
                    sT = p2.tile([128, 1024], F32, tag="sT")
                    for k in range(2):
                        mc = 2 * mc2 + k
                        nc.tensor.matmul(
                            sT[:, k * 512 : (k + 1) * 512],
                            fT[:, mc * 128 : (mc + 1) * 128],
                            gT[:, nb * 512 : (nb + 1) * 512],
                            start=True, stop=True,
                        )
                    eT = et_pool.tile([128, 1024], BF16, tag="eT")
                    if dve_exp_every and mc2 % dve_exp_every == dve_exp_every - 1:
                        nc.vector.tensor_scalar(
                            eT[:].bitcast(I16), sT[:], SCH_A, SCH_B,
                            mybir.AluOpType.mult, mybir.AluOpType.add,
                        )
                    else:
                        nc.scalar.activation(
                            eT[:], sT[:], mybir.ActivationFunctionType.Exp,
                        )
                    for k in range(2):
                        mc = 2 * mc2 + k
                        nc.tensor.matmul(
                            oT[:],
                            h_aug[:, mc * 33 : (mc + 1) * 33],
                            eT[:, k * 512 : (k + 1) * 512],
                            start=(mc == 0), stop=(mc == MC - 1),
                        )
                oT_sb = stage.tile([33, 512], BF16, tag="oT_sb")
                nc.vector.tensor_copy(oT_sb[:], oT[:])
                se_f32 = stage.tile([1, 512], F32, tag="se_f32")
                nc.vector.tensor_copy(se_f32[:], oT[32:33, :])
                pv = p1.tile([C, 512], F32, tag="oT")
                nc.tensor.matmul(
                    pv[:], wv_sb[:], oT_sb[:], start=True, stop=True,
                )
                nc.vector.tensor_copy(vT[:, nb * 512 : (nb + 1) * 512], pv[:])
                for q in range(4):
                    pt = p1.tile([128, 1], F32, tag="oT")
                    nc.tensor.matmul(
                        pt[:],
                        se_f32[0:1, q * 128 : (q + 1) * 128],
                        ones_sb[:],
                        start=True, stop=True,
                    )
                    nc.vector.reciprocal(
                        inv_nat[:, nb * 4 + q : nb * 4 + q + 1], pt[:]
                    )
                for q in range(4):
                    vc = nb * 4 + q
                    pvn = p1.tile([128, C], F32, tag="oT")
                    nc.tensor.matmul(
                        pvn[:], vT[:, vc * 128 : (vc + 1) * 128],
                        id_sb[0:64, 0:64], start=True, stop=True,
                    )
                    nc.vector.tensor_scalar(
                        v_nat[:, vc * C : (vc + 1) * C], pvn[:],
                        inv_nat[:, vc : vc + 1], None,
                        mybir.AluOpType.mult,
                    )
                nc.sync.dma_start(
                    vsc_w[:, 4 * nb : 4 * nb + 4, :],
                    v_nat[:, nb * 256 : (nb + 1) * 256].rearrange(
                        "p (vc c) -> p vc c", vc=4
                    ),
                )

            if phases < 4:
                return nc

            v_blk = persist.tile([128, 4096], F32)
            vsc_r = vscratch.rearrange("(u rl) c -> u (rl c)", u=64)
            nc.sync.dma_start(v_blk[0:64, :], vsc_r)
            nc.sync.dma_start(v_blk[64:128, :], vsc_r)

            x_hb = [
                x.rearrange("(a hb2 b l) c -> hb2 a b (l c)",
                            a=16, hb2=2, b=4, l=256)[h]
                for h in range(2)
            ]
            out_hb = [
                out.rearrange("(a hb2 b l) c -> hb2 a b (l c)",
                              a=16, hb2=2, b=4, l=256)[h]
                for h in range(2)
            ]
            for q in range(4):
                xt = xres_pool.tile([128, 4096], F32, tag="xres")
                for h in range(2):
                    nc.sync.dma_start(
                        xt[h * 64 : (h + 1) * 64, :],
                        x_hb[h][:, :, q * 4096 : (q + 1) * 4096],
                    )
                ot = ores_pool.tile([128, 4096], F32, tag="ores")
                v_b = v_blk[:, q * 1024 : (q + 1) * 1024].rearrange(
                    "p (d2 uu c) -> p d2 uu c", d2=16, uu=1
                ).broadcast_to([128, 16, 2, C])
                for wb in range(2):
                    eng = nc.gpsimd if (gpsimd_resid and wb == 0) else nc.vector
                    eng.tensor_tensor(
                        ot[:, wb * 2048 : (wb + 1) * 2048].rearrange(
                            "p (d2 db c) -> p d2 db c", d2=16, db=2
                        ),
                        xt[:, wb * 2048 : (wb + 1) * 2048].rearrange(
                            "p (d2 db c) -> p d2 db c", d2=16, db=2
                        ),
                        v_b,
                        mybir.AluOpType.add,
                    )
                for h in range(2):
                    nc.sync.dma_start(
                        out_hb[h][:, :, q * 4096 : (q + 1) * 4096],
                        ot[h * 64 : (h + 1) * 64, :],
                    )

    return nc


_CACHE = {}


def _get_nc():
    if "nc" not in _CACHE:
        _CACHE["nc"] = build_kernel()
    return _CACHE["nc"]


def _make_in_maps(inputs):
    import ml_dtypes

    bf16 = ml_dtypes.bfloat16
    x = np.asarray(inputs["x"], dtype=np.float32)
    gamma_v = float(np.asarray(inputs["gamma"]).reshape(-1)[0])
    wf_aug = np.concatenate(
        [np.asarray(inputs["Wf"]), np.asarray(inputs["bf"])[None, :]], 0
    ).astype(np.float32)
    wg_aug = np.concatenate(
        [np.asarray(inputs["Wg"]), np.asarray(inputs["bg"])[None, :]], 0
    ).astype(np.float32)
    wh_aug = np.zeros((65, 33), np.float32)
    wh_aug[:64, :32] = np.asarray(inputs["Wh"])
    wh_aug[64, :32] = np.asarray(inputs["bh"])
    wh_aug[64, 32] = 1.0
    wv_aug = np.concatenate(
        [np.asarray(inputs["Wv"]), np.asarray(inputs["bv"])[None, :]], 0
    ).astype(np.float32) * gamma_v
    shared = {
        "wf": wf_aug.astype(bf16),
        "wg": wg_aug.astype(bf16),
        "wh": wh_aug.astype(bf16),
        "wv": wv_aug.astype(np.float32).astype(bf16),
        "ident": np.eye(128, dtype=np.float32),
    }
    return [
        dict(shared, x=np.ascontiguousarray(x[b].reshape(NROWS, C)))
        for b in range(B)
    ]


def kernel(x, Wf, bf, Wg, bg, Wh, bh, Wv, bv, gamma):
    nc = _get_nc()
    in_maps = _make_in_maps(dict(
        x=x, Wf=Wf, bf=bf, Wg=Wg, bg=bg, Wh=Wh, bh=bh, Wv=Wv, bv=bv,
        gamma=gamma,
    ))
    res = run_bass_kernel_spmd(nc, in_maps, list(range(B)))
    outs = [res.results[b]["out"].reshape(HH, HH, HH, C) for b in range(B)]
    return np.stack(outs).astype(np.float32)


if __name__ == "__main__":
    import reference

    inputs = {k: np.asarray(v) for k, v in reference.setup_inputs().items()}
    got = kernel(**inputs)
    exp = np.asarray(reference.reference(**inputs))
    err = np.abs(got - exp).max() / (np.abs(exp).max() + 1e-30)
    print("Relative error:", err)


# revision 33
# speedup vs baseline: 1.0217x; 1.0217x over previous
import numpy as np

import concourse.bass as bass
import concourse.mybir as mybir
import concourse.tile as tile
from concourse.bass_utils import run_bass_kernel_spmd
from concourse.vector_clock import ScopedClock

_MAX_WAITS = 1


def _split_drain_and_barrier(self, tick_clock, wait_clock):
    import bass_rust

    drain_inst = self.nc.sync.drain()
    wait_clock.add_sem_waits(
        drain_inst.ins, ScopedClock({None: tick_clock.global_clock})
    )
    si = drain_inst.ins.sync_info
    waits = list(si.on_wait)
    if len(waits) > _MAX_WAITS:
        si.on_wait = waits[:_MAX_WAITS]
        drain_inst.ins.sync_info = si
        for k in range(_MAX_WAITS, len(waits), _MAX_WAITS):
            extra = self.nc.sync.drain()
            esi = extra.ins.sync_info
            if esi is None:
                esi = bass_rust.SyncInfo(
                    on_wait=waits[k : k + _MAX_WAITS], on_update=[]
                )
            else:
                esi.on_wait = waits[k : k + _MAX_WAITS]
            extra.ins.sync_info = esi

    self.nc.all_engine_barrier()
    assert self.sems is not None
    popped = self.nc._tile_sem_poison_stack.pop()
    assert popped is self._sem_poison
    self.nc.clear_and_free_semaphores(list(self.sems.allocated().values()))
    self.nc.all_engine_barrier()


tile.TileContext._drain_and_barrier = _split_drain_and_barrier

_orig_lower_ordered = tile.TileContext._lower_ordered_insts


def _split_waits_lower(self, ordered):
    import bass_rust

    for bb, insts in ordered.items():
        new = []
        for inst in insts:
            si = getattr(inst, "sync_info", None)
            waits = list(si.on_wait) if si is not None else []
            if len(waits) > _MAX_WAITS:
                eng = inst.engine
                for w in waits[:-_MAX_WAITS]:
                    carrier = self.nc.engines[eng].drain(fusable=False).ins
                    csi = carrier.sync_info
                    if csi is None:
                        csi = bass_rust.SyncInfo(on_wait=[w], on_update=[])
                    else:
                        csi.on_wait = [w]
                    carrier.sync_info = csi
                    new.append(carrier)
                si.on_wait = waits[-_MAX_WAITS:]
                inst.sync_info = si
            new.append(inst)
        insts[:] = new
    return _orig_lower_ordered(self, ordered)


tile.TileContext._lower_ordered_insts = _split_waits_lower

F32 = mybir.dt.float32
I16 = mybir.dt.int16
BF16 = mybir.dt.bfloat16

B = 8
HH = 32
N = 4096
C = 64
NROWS = 32768
NB = 8
MC = 32

SCH_A = 184.6650
SCH_B = 16248.58

DVE_EXP_EVERY = 3
GPSIMD_RESID = True


def build_kernel(dve_exp_every=DVE_EXP_EVERY, gpsimd_resid=GPSIMD_RESID,
                 phases=99):
    nc = bass.Bass()

    x = nc.declare_dram_parameter("x", [NROWS, C], F32, isOutput=False)
    wf = nc.declare_dram_parameter("wf", [65, 8], BF16, isOutput=False)
    wg = nc.declare_dram_parameter("wg", [65, 8], BF16, isOutput=False)
    wh = nc.declare_dram_parameter("wh", [65, 33], BF16, isOutput=False)
    wv = nc.declare_dram_parameter("wv", [33, C], BF16, isOutput=False)
    ident = nc.declare_dram_parameter("ident", [128, 128], F32, isOutput=False)
    out = nc.declare_dram_parameter("out", [NROWS, C], F32, isOutput=True)

    x_sub = x.rearrange(
        "(h2 hb w2 wb d2 db) c -> hb wb db h2 w2 d2 c",
        h2=16, hb=2, w2=16, wb=2, d2=16, db=2,
    )[0, 0, 0]

    def xs_chunk(mc):
        return x_sub[mc >> 1, 8 * (mc & 1) : 8 * (mc & 1) + 8]

    with tile.TileContext(nc) as tc:
        with (
            tc.tile_pool(name="const", bufs=1) as const_pool,
            tc.tile_pool(name="persist", bufs=1) as persist,
            tc.tile_pool(name="stage", bufs=6) as stage,
            tc.tile_pool(name="et", bufs=6) as et_pool,
            tc.tile_pool(name="xres", bufs=4) as xres_pool,
            tc.tile_pool(name="ores", bufs=2) as ores_pool,
            tc.tile_pool(name="p2", bufs=3, space=bass.MemorySpace.PSUM) as p2,
            tc.tile_pool(name="p1", bufs=2, space=bass.MemorySpace.PSUM) as p1,
        ):
            id_sb = const_pool.tile([128, 128], F32)
            nc.sync.dma_start(id_sb[:], ident[:])
            wf_sb = const_pool.tile([65, 8], BF16)
            nc.sync.dma_start(wf_sb[:], wf[:])
            wg_sb = const_pool.tile([65, 8], BF16)
            nc.sync.dma_start(wg_sb[:], wg[:])
            wh_sb = const_pool.tile([65, 33], BF16)
            nc.sync.dma_start(wh_sb[:], wh[:])
            wv_sb = const_pool.tile([33, C], BF16)
            nc.sync.dma_start(wv_sb[:], wv[:])
            ones_sb = const_pool.tile([1, 1], F32)
            nc.vector.memset(ones_sb[:], 1.0)

            xsT = persist.tile([65, N], BF16)
            nc.vector.memset(xsT[64:65, :], 1.0)
            fT = persist.tile([8, N], BF16)
            gT = persist.tile([8, N], BF16)
            h_aug = persist.tile([128, MC * 33], BF16)
            for mc in range(MC):
                st = stage.tile([128, C], F32, tag="xs_stage")
                nc.sync.dma_start(st[:], xs_chunk(mc))
                tp = p1.tile([64, 128], F32, tag="oT")
                nc.tensor.matmul(tp[:], st[:], id_sb[:], start=True, stop=True)
                nc.scalar.copy(xsT[0:64, mc * 128 : (mc + 1) * 128], tp[:])
                ph = p1.tile([128, 33], F32, tag="oT")
                nc.tensor.matmul(
                    ph[:], xsT[:, mc * 128 : (mc + 1) * 128], wh_sb[:],
                    start=True, stop=True,
                )
                nc.scalar.copy(h_aug[:, mc * 33 : (mc + 1) * 33], ph[:])
                if mc % 4 == 3:
                    nb = mc // 4
                    pf = p1.tile([8, 512], F32, tag="oT")
                    nc.tensor.matmul(
                        pf[:], wf_sb[:], xsT[:, nb * 512 : (nb + 1) * 512],
                        start=True, stop=True,
                    )
                    nc.vector.tensor_copy(
                        fT[:, nb * 512 : (nb + 1) * 512], pf[:]
                    )
                    pg = p1.tile([8, 512], F32, tag="oT")
                    nc.tensor.matmul(
                        pg[:], wg_sb[:], xsT[:, nb * 512 : (nb + 1) * 512],
                        start=True, stop=True,
                    )
                    nc.vector.tensor_copy(
                        gT[:, nb * 512 : (nb + 1) * 512], pg[:]
                    )

            if phases < 3:
                return nc

            vT = persist.tile([C, N], F32)
            inv_nat = persist.tile([128, MC], F32)
            v_nat = persist.tile([128, MC * C], F32)
            vscratch = nc.dram_tensor("vscratch", [N, C], F32)
            vsc_w = vscratch.rearrange("(vc p) c -> p vc c", p=128)

            for nb in range(NB):
                oT = p1.tile([33, 512], F32, tag="oT")
                for mc2 in range(MC // 2):
                    sT = p2.tile([128, 1024], F32, tag="sT")
                    for k in range(2):
                        mc = 2 * mc2 + k
                        nc.tensor.matmul(
                            sT[:, k * 512 : (k + 1) * 512],
                            fT[:, mc * 128 : (mc + 1) * 128],
                            gT[:, nb * 512 : (nb + 1) * 512],
                            start=True, stop=True,
                        )
                    eT = et_pool.tile([128, 1024], BF16, tag="eT")
                    if dve_exp_every and mc2 % dve_exp_every == dve_exp_every - 1:
                        nc.vector.tensor_scalar(
                            eT[:].bitcast(I16), sT[:], SCH_A, SCH_B,
                            mybir.AluOpType.mult, mybir.AluOpType.add,
                        )
                    else:
                        nc.scalar.activation(
                            eT[:], sT[:], mybir.ActivationFunctionType.Exp,
                        )
                    for k in range(2):
                        mc = 2 * mc2 + k
                        nc.tensor.matmul(
                            oT[:],
                            h_aug[:, mc * 33 : (mc + 1) * 33],
                            eT[:, k * 512 : (k + 1) * 512],
                            start=(mc == 0), stop=(mc == MC - 1),
                        )
                oT_sb = stage.tile([33, 512], BF16, tag="oT_sb")
                nc.vector.tensor_copy(oT_sb[:], oT[:])
                se_f32 = stage.tile([1, 512], F32, tag="se_f32")
                nc.vector.tensor_copy(se_f32[:], oT[32:33, :])
                pv = p1.tile([C, 512], F32, tag="oT")
                nc.tensor.matmul(
                    pv[:], wv_sb[:], oT_sb[:], start=True, stop=True,
                )
                nc.vector.tensor_copy(vT[:, nb * 512 : (nb + 1) * 512], pv[:])
                for q in range(4):
                    pt = p1.tile([128, 1], F32, tag="oT")
                    nc.tensor.matmul(
                        pt[:],
                        se_f32[0:1, q * 128 : (q + 1) * 128],
                        ones_sb[:],
                        start=True, stop=True,
                    )
                    nc.vector.reciprocal(
                        inv_nat[:, nb * 4 + q : nb * 4 + q + 1], pt[:]
                    )
                for q in range(4):
                    vc = nb * 4 + q
                    pvn = p1.tile([128, C], F32, tag="oT")
                    nc.tensor.matmul(
                        pvn[:], vT[:, vc * 128 : (vc + 1) * 128],
                        id_sb[0:64, 0:64], start=True, stop=True,
                    )
                    nc.vector.tensor_scalar(
                        v_nat[:, vc * C : (vc + 1) * C], pvn[:],
                        inv_nat[:, vc : vc + 1], None,
                        mybir.AluOpType.mult,
                    )
                nc.sync.dma_start(
                    vsc_w[:, 4 * nb : 4 * nb + 4, :],
                    v_nat[:, nb * 256 : (nb + 1) * 256].rearrange(
                        "p (vc c) -> p vc c", vc=4
                    ),
                )

            if phases < 4:
                return nc

            v_blk = persist.tile([128, 4096], F32)
            vsc_r = vscratch.rearrange("(u rl) c -> u (rl c)", u=64)
            nc.sync.dma_start(v_blk[0:64, :], vsc_r)
            nc.sync.dma_start(v_blk[64:128, :], vsc_r)

            x_hb = [
                x.rearrange("(a hb2 b l) c -> hb2 a b (l c)",
                            a=16, hb2=2, b=4, l=256)[h]
                for h in range(2)
            ]
            out_hb = [
                out.rearrange("(a hb2 b l) c -> hb2 a b (l c)",
                              a=16, hb2=2, b=4, l=256)[h]
                for h in range(2)
            ]
            for q in range(4):
                xt = xres_pool.tile([128, 4096], F32, tag="xres")
                for h in range(2):
                    nc.sync.dma_start(
                        xt[h * 64 : (h + 1) * 64, :],
                        x_hb[h][:, :, q * 4096 : (q + 1) * 4096],
                    )
                ot = ores_pool.tile([128, 4096], F32, tag="ores")
                v_b = v_blk[:, q * 1024 : (q + 1) * 1024].rearrange(
                    "p (d2 uu c) -> p d2 uu c", d2=16, uu=1
                ).broadcast_to([128, 16, 2, C])
                for wb in range(2):
                    eng = nc.gpsimd if (gpsimd_resid and wb == 0) else nc.vector
                    eng.tensor_tensor(
                        ot[:, wb * 2048 : (wb + 1) * 2048].rearrange(
                            "p (d2 db c) -> p d2 db c", d2=16, db=2
                        ),
                        xt[:, wb * 2048 : (wb + 1) * 2048].rearrange(
                            "p (d2 db c) -> p d2 db c", d2=16, db=2
                        ),
                        v_b,
                        mybir.AluOpType.add,
                    )
                for h in range(2):
                    nc.sync.dma_start(
                        out_hb[h][:, :, q * 4096 : (q + 1) * 4096],
                        ot[h * 64 : (h + 1) * 64, :],
                    )

    return nc


_CACHE = {}


def _get_nc():
    if "nc" not in _CACHE:
        _CACHE["nc"] = build_kernel()
    return _CACHE["nc"]


def _make_in_maps(inputs):
    import ml_dtypes

    bf16 = ml_dtypes.bfloat16
    x = np.asarray(inputs["x"], dtype=np.float32)
    gamma_v = float(np.asarray(inputs["gamma"]).reshape(-1)[0])
    wf_aug = np.concatenate(
        [np.asarray(inputs["Wf"]), np.asarray(inputs["bf"])[None, :]], 0
    ).astype(np.float32)
    wg_aug = np.concatenate(
        [np.asarray(inputs["Wg"]), np.asarray(inputs["bg"])[None, :]], 0
    ).astype(np.float32)
    wh_aug = np.zeros((65, 33), np.float32)
    wh_aug[:64, :32] = np.asarray(inputs["Wh"])
    wh_aug[64, :32] = np.asarray(inputs["bh"])
    wh_aug[64, 32] = 1.0
    wv_aug = np.concatenate(
        [np.asarray(inputs["Wv"]), np.asarray(inputs["bv"])[None, :]], 0
    ).astype(np.float32) * gamma_v
    shared = {
        "wf": wf_aug.astype(bf16),
        "wg": wg_aug.astype(bf16),
        "wh": wh_aug.astype(bf16),
        "wv": wv_aug.astype(np.float32).astype(bf16),
        "ident": np.eye(128, dtype=np.float32),
    }
    return [
        dict(shared, x=np.ascontiguousarray(x[b].reshape(NROWS, C)))
        for b in range(B)
    ]


def kernel(x, Wf, bf, Wg, bg, Wh, bh, Wv, bv, gamma):
    nc = _get_nc()
    in_maps = _make_in_maps(dict(
        x=x, Wf=Wf, bf=bf, Wg=Wg, bg=bg, Wh=Wh, bh=bh, Wv=Wv, bv=bv,
        gamma=gamma,
    ))
    res = run_bass_kernel_spmd(nc, in_maps, list(range(B)))
    outs = [res.results[b]["out"].reshape(HH, HH, HH, C) for b in range(B)]
    return np.stack(outs).astype(np.float32)


if __name__ == "__main__":
    import reference

    inputs = {k: np.asarray(v) for k, v in reference.setup_inputs().items()}
    got = kernel(**inputs)
    exp = np.asarray(reference.reference(**inputs))
    err = np.abs(got - exp).max() / (np.abs(exp).max() + 1e-30)
    print("Relative error:", err)
